# revision 1
# baseline (speedup 1.0000x reference)
"""DeepSetPred Trainium2 kernel: 3-layer token encoder MLP + segment-sum +
predictor MLP on 8 NeuronCores, with ZERO collectives.

Sharding: the host cuts the (sorted-by-segment) token axis at segment
boundaries, so every segment belongs to exactly one core. Each shard is
padded to a common length with tokens whose one-hot selector row is all
zero (they flow through the encoder but contribute nothing to any segment).
Each core therefore computes the complete segment sums for its own
contiguous range of <=32 segments, runs the predictor on just those rows,
and writes its private slice of the output; the host concatenates.

Layout: tokens on the matmul free dim (features on partitions), fp16
encoder matmuls (2-byte fast weight loads, fp32 PSUM accumulation,
~7e-4 rel err), per-feature bias+tanh fused on the ScalarEngine, the
ragged segment-sum as a one-hot stationary matmul accumulating into a
persistent PSUM bank with the n_s*b3 bias folded in as one K=1 fp32
matmul, and an fp32r predictor.
"""

import numpy as np

import concourse.mybir as mybir
import concourse.tile as tile
from concourse import bacc
from concourse import bass_utils
from concourse.masks import make_identity

# Problem shapes (hardcoded per contract).
T, E, H, C, O = 131072, 256, 512, 256, 32
S = 128            # num segments
N_CORES = 8
TOK = 512          # tokens per inner chunk
G = 1              # chunks per super-chunk (DMA batching granularity)
SCTOK = G * TOK    # 1024
MIN_SLOTS = 32     # baseline segments-per-core capacity
F32 = mybir.dt.float32
F32R = mybir.dt.float32r
F16 = mybir.dt.float16

_CACHE = {}


def _mm(nc, out, lhsT, rhs, start, stop, skip=False):
    nc.tensor.matmul(out, lhsT, rhs,
                     start=start, stop=stop, skip_group_check=skip)


def _build_nc(t_sh, SLOTS):
    assert t_sh % 128 == 0

    nc = bacc.Bacc("TRN2", target_bir_lowering=False, debug=False,
                   num_devices=N_CORES)

    xt_d = nc.dram_tensor("xt", [E, t_sh], F16, kind="ExternalInput")
    sel_d = nc.dram_tensor("sel", [t_sh, SLOTS], F16, kind="ExternalInput")
    cnt_d = nc.dram_tensor("cnt", [1, SLOTS], F32, kind="ExternalInput")
    w1_d = nc.dram_tensor("w1", [E, H], F16, kind="ExternalInput")
    w2_d = nc.dram_tensor("w2", [H, H], F16, kind="ExternalInput")
    w3_d = nc.dram_tensor("w3", [H, C], F16, kind="ExternalInput")
    b1_d = nc.dram_tensor("b1", [H // 128, 128], F32, kind="ExternalInput")
    b2_d = nc.dram_tensor("b2", [H // 128, 128], F32, kind="ExternalInput")
    b3_d = nc.dram_tensor("b3", [1, C], F32, kind="ExternalInput")
    p1_d = nc.dram_tensor("p1", [C, H], F32R, kind="ExternalInput")
    p2_d = nc.dram_tensor("p2", [H, H], F32R, kind="ExternalInput")
    p3_d = nc.dram_tensor("p3", [H, O], F32R, kind="ExternalInput")
    pb1_d = nc.dram_tensor("pb1", [H // 128, 128], F32, kind="ExternalInput")
    pb2_d = nc.dram_tensor("pb2", [H // 128, 128], F32, kind="ExternalInput")
    pb3_d = nc.dram_tensor("pb3", [1, O], F32, kind="ExternalInput")
    out_d = nc.dram_tensor("pred", [SLOTS, O], F32, kind="ExternalOutput")

    EC = E // 128   # 2
    HC = H // 128   # 4
    CC = C // 128   # 2
    TT = TOK // 128  # 4 token sub-tiles per chunk

    with tile.TileContext(nc) as tc:
        with tc.tile_pool(name="wts", bufs=1) as wp, \
             tc.tile_pool(name="xt", bufs=3) as xtp, \
             tc.tile_pool(name="sel", bufs=3) as selp, \
             tc.tile_pool(name="act", bufs=4) as actp, \
             tc.tile_pool(name="small", bufs=1) as smp, \
             tc.tile_pool(name="ps", bufs=2, space="PSUM") as psp, \
             tc.tile_pool(name="psacc", bufs=1, space="PSUM") as psa:

            # warm the ACT tanh table before the scalar queue fills with DMAs
            warm_sb = smp.tile([1, 1], F32, tag="warm", name="warm")
            nc.gpsimd.memset(warm_sb[:], 0.0)
            warm_o = smp.tile([1, 1], F32, tag="warmo", name="warmo")
            nc.scalar.activation(warm_o[:], warm_sb[:],
                                 mybir.ActivationFunctionType.Tanh)

            # ---- resident weights (one batched HWDGE DMA per matrix;
            # encoder weights on the scalar ring, predictor weights on
            # gpsimd so the ACT queue stays clear for tanh) ----
            w1_t = wp.tile([128, EC, HC, 128], F16, tag="w1", name="w1t")
            nc.scalar.dma_start(
                w1_t[:], w1_d.ap().rearrange("(e p) (h q) -> p e h q",
                                             p=128, q=128))
            w1_sb = [[w1_t[:, e, h, :] for h in range(HC)] for e in range(EC)]
            w2_t = wp.tile([128, HC, HC, 128], F16, tag="w2", name="w2t")
            nc.scalar.dma_start(
                w2_t[:], w2_d.ap().rearrange("(k p) (h q) -> p k h q",
                                             p=128, q=128))
            w2_sb = [[w2_t[:, k, h, :] for h in range(HC)] for k in range(HC)]
            w3_t = wp.tile([128, HC, C], F16, tag="w3", name="w3t")
            nc.scalar.dma_start(
                w3_t[:], w3_d.ap().rearrange("(k p) c -> p k c", p=128))
            w3_sb = [w3_t[:, k, :] for k in range(HC)]
            # ---- biases / rows (gpsimd ring; tiny) ----
            b1_sb = smp.tile([128, HC], F32, tag="b1", name="b1")
            nc.gpsimd.dma_start(b1_sb[:], b1_d.ap().rearrange("h p -> p h"))
            b2_sb = smp.tile([128, HC], F32, tag="b2", name="b2")
            nc.gpsimd.dma_start(b2_sb[:], b2_d.ap().rearrange("h p -> p h"))
            pb1_sb = smp.tile([128, HC], F32, tag="pb1", name="pb1")
            nc.gpsimd.dma_start(pb1_sb[:], pb1_d.ap().rearrange("h p -> p h"))
            pb2_sb = smp.tile([128, HC], F32, tag="pb2", name="pb2")
            nc.gpsimd.dma_start(pb2_sb[:], pb2_d.ap().rearrange("h p -> p h"))
            b3row = smp.tile([1, C], F32, tag="b3row", name="b3row")
            nc.gpsimd.dma_start(b3row[:], b3_d.ap())
            pb3row = smp.tile([1, O], F32, tag="pb3row", name="pb3row")
            nc.gpsimd.dma_start(pb3row[:], pb3_d.ap())
            cntrow = smp.tile([1, SLOTS], F32, tag="cntrow", name="cntrow")
            nc.gpsimd.dma_start(cntrow[:], cnt_d.ap())
            ones1 = smp.tile([1, SLOTS], F32, tag="ones1", name="ones1")
            nc.gpsimd.memset(ones1[:], 1.0)
            ident = smp.tile([SLOTS, SLOTS], F32, tag="ident", name="ident")
            make_identity(nc, ident[:])

            p1_t = wp.tile([128, CC, HC, 128], F32R, tag="p1", name="p1t")
            nc.gpsimd.dma_start(
                p1_t[:], p1_d.ap().rearrange("(c p) (h q) -> p c h q",
                                             p=128, q=128))
            p1_sb = [[p1_t[:, c, h, :] for h in range(HC)] for c in range(CC)]
            p2_t = wp.tile([128, HC, HC, 128], F32R, tag="p2", name="p2t")
            nc.gpsimd.dma_start(
                p2_t[:], p2_d.ap().rearrange("(k p) (h q) -> p k h q",
                                             p=128, q=128))
            p2_sb = [[p2_t[:, k, h, :] for h in range(HC)] for k in range(HC)]
            p3_t = wp.tile([128, HC, O], F32R, tag="p3", name="p3t")
            nc.gpsimd.dma_start(
                p3_t[:], p3_d.ap().rearrange("(k p) o -> p k o", p=128))
            p3_sb = [p3_t[:, k, :] for k in range(HC)]

            # ---- persistent segment-sum accumulator enc[slot, c] ----
            enc_ps = psa.tile([SLOTS, C], F32, tag="encacc", name="encacc")
            # enc[slot, c] = counts[slot] * b3[c]  (K=1 fp32 matmul opens it)
            nc.tensor.matmul(enc_ps[:], cntrow[:], b3row[:],
                             start=True, stop=False, skip_group_check=True)

            # ---- main token loop, software-pipelined with a 1-chunk skew:
            # L1(i+1) is emitted before L2/L3/seg(i) so the PE's strict-FIFO
            # queue never head-of-line blocks on the tanh chain ----
            n_full = t_sh // TOK
            tail = t_sh - n_full * TOK
            chunks = [(i * TOK, TOK) for i in range(n_full)]
            if tail:
                chunks.append((n_full * TOK, tail))

            def load_and_l1(base, tok):
                xt_t = xtp.tile([128, EC, tok], F16, tag="xt", name="xt",
                                padded_shape=[128, EC, TOK])
                nc.sync.dma_start(
                    xt_t[:],
                    xt_d.ap()[:, base:base + tok]
                        .rearrange("(e p) t -> p e t", p=128))
                sel_t = selp.tile([128, tok // 128, SLOTS], F16, tag="sel",
                                  name="sel", padded_shape=[128, TT, SLOTS])
                nc.sync.dma_start(
                    sel_t[:],
                    sel_d.ap()[base:base + tok, :]
                         .rearrange("(q p) s -> p q s", p=128))
                h1_t = actp.tile([128, HC, tok], F16, tag="h1", name="h1",
                                 bufs=5, padded_shape=[128, HC, TOK])
                for h in range(HC):
                    ps1 = psp.tile([128, tok], F32, tag="mm", name="mm",
                                   bufs=4, padded_shape=[128, TOK])
                    for e in range(EC):
                        _mm(nc, ps1[:], w1_sb[e][h], xt_t[:, e, :],
                            start=(e == 0), stop=(e == EC - 1), skip=True)
                    nc.scalar.activation(h1_t[:, h, :], ps1[:],
                                         mybir.ActivationFunctionType.Tanh,
                                         bias=b1_sb[:, h:h + 1])
                return sel_t, h1_t

            def l2_l3_seg(sel_t, h1_t, tok, is_last):
                tt = tok // 128
                h2_t = actp.tile([128, HC, tok], F16, tag="h2", name="h2",
                                 padded_shape=[128, HC, TOK])
                for h in range(HC):
                    ps2 = psp.tile([128, tok], F32, tag="mm", name="mm",
                                   bufs=4, padded_shape=[128, TOK])
                    for k in range(HC):
                        _mm(nc, ps2[:], w2_sb[k][h], h1_t[:, k, :],
                            start=(k == 0), stop=(k == HC - 1), skip=True)
                    nc.scalar.activation(h2_t[:, h, :], ps2[:],
                                         mybir.ActivationFunctionType.Tanh,
                                         bias=b2_sb[:, h:h + 1])
                te_sb = actp.tile([128, tt, C], F16, tag="te", name="te",
                                  padded_shape=[128, TT, C])
                for t in range(tt):
                    ps3 = psp.tile([128, C], F32, tag="l3", name="l3", bufs=3)
                    for k in range(HC):
                        _mm(nc, ps3[:], h2_t[:, k, t * 128:(t + 1) * 128],
                            w3_sb[k], start=(k == 0), stop=(k == HC - 1))
                    nc.vector.tensor_copy(te_sb[:, t, :], ps3[:])
                for t in range(tt):
                    last = is_last and (t == tt - 1)
                    _mm(nc, enc_ps[:], sel_t[:, t, :], te_sb[:, t, :],
                        start=False, stop=last, skip=True)

            pend = []
            for ci, (base, tok) in enumerate(chunks):
                pend.append(load_and_l1(base, tok) + (tok,))
                keep = 2 if ci < 2 else 1
                while len(pend) > keep:
                    l2_l3_seg(*pend.pop(0), is_last=False)
            while pend:
                args = pend.pop(0)
                l2_l3_seg(*args, is_last=(len(pend) == 0))

            # ---- predictor on this core's own <=SLOTS segment rows ----
            enc_sb = smp.tile([SLOTS, C], F32, tag="encsb", name="encsb")
            nc.vector.tensor_copy(enc_sb[:], enc_ps[:])
            encT_sb = smp.tile([128, CC, SLOTS], F32R, tag="encT", name="encT")
            for c in range(CC):
                pst = psp.tile([128, SLOTS], F32, tag="l3", name="pst", bufs=3)
                nc.tensor.transpose(pst[:], enc_sb[:, c * 128:(c + 1) * 128],
                                    ident[:])
                nc.vector.tensor_copy(encT_sb[:, c, :], pst[:])

            q1_sb = smp.tile([128, HC, SLOTS], F32R, tag="q1", name="q1")
            for h in range(HC):
                pp1 = psp.tile([128, SLOTS], F32, tag="mm", name="pp1", bufs=4)
                for c in range(CC):
                    _mm(nc, pp1[:], p1_sb[c][h], encT_sb[:, c, :],
                        start=(c == 0), stop=(c == CC - 1))
                nc.scalar.activation(q1_sb[:, h, :], pp1[:],
                                     mybir.ActivationFunctionType.Tanh,
                                     bias=pb1_sb[:, h:h + 1])
            q2_sb = smp.tile([128, HC, SLOTS], F32R, tag="q2", name="q2")
            for h in range(HC):
                pp2 = psp.tile([128, SLOTS], F32, tag="mm", name="pp2", bufs=4)
                for k in range(HC):
                    _mm(nc, pp2[:], p2_sb[k][h], q1_sb[:, k, :],
                        start=(k == 0), stop=(k == HC - 1))
                nc.scalar.activation(q2_sb[:, h, :], pp2[:],
                                     mybir.ActivationFunctionType.Tanh,
                                     bias=pb2_sb[:, h:h + 1])

            # final: pred[slot, o] = q2.T @ P3 + pb3
            ppo = psp.tile([SLOTS, O], F32, tag="l3", name="ppo", bufs=3)
            nc.tensor.matmul(ppo[:], ones1[:], pb3row[:],
                             start=True, stop=False, skip_group_check=True)
            for k in range(HC):
                _mm(nc, ppo[:], q2_sb[:, k, :], p3_sb[k],
                    start=False, stop=(k == HC - 1), skip=True)
            pred_sb = smp.tile([SLOTS, O], F32, tag="pred", name="predsb")
            nc.vector.tensor_copy(pred_sb[:], ppo[:])
            nc.sync.dma_start(out_d.ap(), pred_sb[:])

    nc.compile()
    return nc


def kernel(words, seg_ids, W1, b1, W2, b2, W3, b3,
           P1, pb1, P2, pb2, P3, pb3, batch_size, alpha_iter, **_):
    words = np.asarray(words, dtype=np.float32)
    seg_ids = np.asarray(seg_ids).astype(np.int64)
    assert words.shape == (T, E), words.shape
    bs, ai = int(batch_size), int(alpha_iter)

    # --- host-side index prep: cut the sorted token axis at segment
    # boundaries so each core owns whole segments ---
    counts = np.bincount(seg_ids, minlength=S)[:S]
    starts = np.concatenate([[0], np.cumsum(counts)])   # [S+1]
    cuts = [0]
    for c in range(1, N_CORES):
        tgt = c * T // N_CORES
        j = int(np.searchsorted(starts, tgt, side="left"))
        if j > 0 and tgt - starts[j - 1] < starts[j] - tgt:
            j -= 1
        cuts.append(int(starts[j]))
    cuts.append(T)
    lens = np.diff(cuts)
    t_sh = int(np.ceil(lens.max() / 128) * 128)

    # contiguous segment range owned by each core (covers all of [0, S));
    # empty shards inherit the following shard's start so ranges stay
    # monotone and collectively exhaustive
    seg_lo = [0] * N_CORES
    for c in range(N_CORES - 1, 0, -1):
        if lens[c] > 0:
            seg_lo[c] = int(seg_ids[cuts[c]])
        else:
            seg_lo[c] = S if c == N_CORES - 1 else seg_lo[c + 1]
    seg_hi = seg_lo[1:] + [S]
    slots_needed = max(seg_hi[c] - seg_lo[c] for c in range(N_CORES))
    SLOTS = min(128, max(MIN_SLOTS, ((slots_needed + 31) // 32) * 32))
    assert slots_needed <= SLOTS, (seg_lo, seg_hi)
    assert bs * ai == S

    xt = np.ascontiguousarray(words.T.astype(np.float16))    # [E, T] fp16

    key = ("nc", t_sh, SLOTS)
    if key not in _CACHE:
        _CACHE[key] = _build_nc(t_sh, SLOTS)
    nc = _CACHE[key]

    common = {
        "w1": np.ascontiguousarray(W1, dtype=np.float16),
        "w2": np.ascontiguousarray(W2, dtype=np.float16),
        "w3": np.ascontiguousarray(W3, dtype=np.float16),
        "b1": np.ascontiguousarray(b1, dtype=np.float32).reshape(H // 128, 128),
        "b2": np.ascontiguousarray(b2, dtype=np.float32).reshape(H // 128, 128),
        "b3": np.ascontiguousarray(b3, dtype=np.float32).reshape(1, C),
        "p1": np.ascontiguousarray(P1, dtype=np.float32),
        "p2": np.ascontiguousarray(P2, dtype=np.float32),
        "p3": np.ascontiguousarray(P3, dtype=np.float32),
        "pb1": np.ascontiguousarray(pb1, dtype=np.float32).reshape(H // 128, 128),
        "pb2": np.ascontiguousarray(pb2, dtype=np.float32).reshape(H // 128, 128),
        "pb3": np.ascontiguousarray(pb3, dtype=np.float32).reshape(1, O),
    }
    in_maps = []
    for c in range(N_CORES):
        lo, hi = cuts[c], cuts[c + 1]
        n = hi - lo
        xt_c = np.zeros((E, t_sh), dtype=np.float16)
        xt_c[:, :n] = xt[:, lo:hi]
        sel_c = np.zeros((t_sh, SLOTS), dtype=np.float16)
        sel_c[:n, :] = (seg_ids[lo:hi, None] ==
                        (seg_lo[c] + np.arange(SLOTS))[None, :])
        cnt_c = np.zeros((1, SLOTS), dtype=np.float32)
        nseg = seg_hi[c] - seg_lo[c]
        cnt_c[0, :nseg] = counts[seg_lo[c]:seg_hi[c]]
        in_maps.append({
            **common,
            "xt": xt_c,
            "sel": sel_c,
            "cnt": cnt_c,
        })

    global _LAST_IN_MAPS
    _LAST_IN_MAPS = in_maps
    res = bass_utils.run_bass_kernel_spmd(nc, in_maps,
                                          core_ids=list(range(N_CORES)))
    pred = np.zeros((S, O), dtype=np.float32)
    for c in range(N_CORES):
        nseg = seg_hi[c] - seg_lo[c]
        if nseg > 0:
            pred[seg_lo[c]:seg_hi[c]] = res.results[c]["pred"][:nseg]
    return pred.reshape(bs, ai, O).astype(np.float32)


_LAST_IN_MAPS = None



# revision 3
# speedup vs baseline: 1.1943x; 1.1943x over previous
"""DeepSetPred Trainium2 kernel: token encoder MLP + segment-sum + predictor
MLP on 8 NeuronCores, zero collectives.

Sharding: the host cuts the (sorted-by-segment) token axis at segment
boundaries, so every segment belongs to exactly one core. Each shard is
padded to a common length with tokens whose one-hot selector row is all
zero. Each core computes the complete segment sums for its own contiguous
range of <=SLOTS segments, runs the predictor on those rows, and writes its
private slice of the output; the host concatenates.

Key restructure vs the v1 kernel: the encoder's third linear layer commutes
with the segment sum (it sits after the last tanh), so
    segsum(h2 @ W3 + b3) == segsum(h2) @ W3 + counts * b3
and W3 additionally folds into the predictor's first layer:
    enc @ P1 + pb1 == segsum(h2) @ (W3 @ P1) + counts * (b3 @ P1) + pb1.
The per-token path is therefore only L1 + L2 + a one-hot segsum matmul over
h2 (14336 PE rows per 512-token chunk instead of 17408). L2 is computed
token-major (h1 tile stationary, W2 moving) so the segsum needs no
transpose; its bias is added by the DVE from a broadcast tile (ACT bias is
per-partition only), then ACT applies tanh. The PE stream is skewed
L1(c) | L2(c-1) | seg(c-2) so the DVE+ACT hop never stalls the PE. All
weights are host-pre-shuffled into dense [128, X] blocks so every DMA is
partition-contiguous.
"""

import numpy as np

import concourse.mybir as mybir
import concourse.tile as tile
from concourse import bacc
from concourse import bass_utils
from concourse.masks import make_identity

# Problem shapes (hardcoded per contract).
T, E, H, C, O = 131072, 256, 512, 256, 32
S = 128            # num segments
N_CORES = 8
TOK = 512          # tokens per chunk
MIN_SLOTS = 32     # baseline segments-per-core capacity
F32 = mybir.dt.float32
F32R = mybir.dt.float32r
F16 = mybir.dt.float16

EC = E // 128   # 2
HC = H // 128   # 4
TT = TOK // 128  # 4 token sub-tiles per chunk

_CACHE = {}


def _mm(nc, out, lhsT, rhs, start, stop, skip=True):
    nc.tensor.matmul(out, lhsT, rhs,
                     start=start, stop=stop, skip_group_check=skip)


def _build_nc(t_sh, SLOTS):
    assert t_sh % 128 == 0
    n_full = t_sh // TOK
    tail = t_sh - n_full * TOK
    chunks = [(i * TOK, TOK) for i in range(n_full)]
    if tail:
        chunks.append((n_full * TOK, tail))
    NCH = len(chunks)

    nc = bacc.Bacc("TRN2", target_bir_lowering=False, debug=False,
                   num_devices=N_CORES)

    xt_d = nc.dram_tensor("xt", [E, t_sh], F16, kind="ExternalInput")
    # sel packed per chunk: [128, NCH, TT, SLOTS] flattened on the free dim
    sel_d = nc.dram_tensor("sel", [128, NCH * TT * SLOTS], F16,
                           kind="ExternalInput")
    cnt_d = nc.dram_tensor("cnt", [1, SLOTS], F32, kind="ExternalInput")
    # dense pre-shuffled weights: [128, ...] partition-major blocks
    w1_d = nc.dram_tensor("w1", [128, EC * HC * 128], F16,
                          kind="ExternalInput")
    w2_d = nc.dram_tensor("w2", [128, HC * H], F16, kind="ExternalInput")
    b1_d = nc.dram_tensor("b1", [128, HC], F32, kind="ExternalInput")
    b2f_d = nc.dram_tensor("b2f", [128, H], F32, kind="ExternalInput")
    wp1_d = nc.dram_tensor("wp1", [128, HC * HC * 128], F32,
                           kind="ExternalInput")   # W3 @ P1, k-major tiles
    b3p1_d = nc.dram_tensor("b3p1", [1, H], F32, kind="ExternalInput")
    p2_d = nc.dram_tensor("p2", [128, HC * HC * 128], F32,
                          kind="ExternalInput")
    p3_d = nc.dram_tensor("p3", [128, HC * O], F32, kind="ExternalInput")
    pb1_d = nc.dram_tensor("pb1", [128, HC], F32, kind="ExternalInput")
    pb2_d = nc.dram_tensor("pb2", [128, HC], F32, kind="ExternalInput")
    pb3_d = nc.dram_tensor("pb3", [1, O], F32, kind="ExternalInput")
    out_d = nc.dram_tensor("pred", [SLOTS, O], F32, kind="ExternalOutput")

    with tile.TileContext(nc) as tc:
        with tc.tile_pool(name="wts", bufs=1) as wp, \
             tc.tile_pool(name="xt", bufs=4) as xtp, \
             tc.tile_pool(name="sel", bufs=6) as selp, \
             tc.tile_pool(name="act", bufs=3) as actp, \
             tc.tile_pool(name="small", bufs=1) as smp, \
             tc.tile_pool(name="ps", bufs=2, space="PSUM") as psp, \
             tc.tile_pool(name="psacc", bufs=1, space="PSUM") as psa:

            # warm the ACT tanh table before the queues fill
            warm_sb = smp.tile([1, 1], F32, tag="warm", name="warm")
            nc.gpsimd.memset(warm_sb[:], 0.0)
            warm_o = smp.tile([1, 1], F32, tag="warmo", name="warmo")
            nc.scalar.activation(warm_o[:], warm_sb[:],
                                 mybir.ActivationFunctionType.Tanh)

            # ---- resident weights; every DMA is partition-contiguous ----
            # sync ring: w1 first (blocks the first L1), then token stream
            w1_t = wp.tile([128, EC, HC, 128], F16, tag="w1", name="w1t")
            nc.sync.dma_start(
                w1_t[:], w1_d.ap().rearrange("p (e h q) -> p e h q",
                                             e=EC, h=HC))
            # scalar ring: L2 weights + biases (needed ~chunk 0 L2)
            w2_t = wp.tile([128, HC, H], F16, tag="w2", name="w2t")
            nc.scalar.dma_start(
                w2_t[:], w2_d.ap().rearrange("p (k j) -> p k j", k=HC))
            b1_sb = smp.tile([128, HC], F32, tag="b1", name="b1")
            nc.scalar.dma_start(b1_sb[:], b1_d.ap())
            b2f_sb = smp.tile([128, H], F32, tag="b2f", name="b2f")
            nc.scalar.dma_start(b2f_sb[:], b2f_d.ap())
            # gpsimd ring: predictor weights (needed only at the end)
            wp1_t = wp.tile([128, HC, HC, 128], F32R, tag="wp1", name="wp1t")
            nc.gpsimd.dma_start(
                wp1_t[:], wp1_d.ap().rearrange("p (k h q) -> p k h q",
                                               k=HC, h=HC))
            p2_t = wp.tile([128, HC, HC, 128], F32R, tag="p2", name="p2t")
            nc.gpsimd.dma_start(
                p2_t[:], p2_d.ap().rearrange("p (k h q) -> p k h q",
                                             k=HC, h=HC))
            p3_t = wp.tile([128, HC, O], F32R, tag="p3", name="p3t")
            nc.gpsimd.dma_start(
                p3_t[:], p3_d.ap().rearrange("p (k o) -> p k o", k=HC))
            b3p1row = smp.tile([1, H], F32, tag="b3p1", name="b3p1")
            nc.gpsimd.dma_start(b3p1row[:], b3p1_d.ap())
            pb1_sb = smp.tile([128, HC], F32, tag="pb1", name="pb1")
            nc.gpsimd.dma_start(pb1_sb[:], pb1_d.ap())
            pb2_sb = smp.tile([128, HC], F32, tag="pb2", name="pb2")
            nc.gpsimd.dma_start(pb2_sb[:], pb2_d.ap())
            pb3row = smp.tile([1, O], F32, tag="pb3row", name="pb3row")
            nc.gpsimd.dma_start(pb3row[:], pb3_d.ap())
            cntrow = smp.tile([1, SLOTS], F32, tag="cntrow", name="cntrow")
            nc.gpsimd.dma_start(cntrow[:], cnt_d.ap())
            ones1 = smp.tile([1, SLOTS], F32, tag="ones1", name="ones1")
            nc.gpsimd.memset(ones1[:], 1.0)
            ident = smp.tile([SLOTS, SLOTS], F32, tag="ident", name="ident")
            make_identity(nc, ident[:])

            # ---- persistent segment-sum accumulator Z[slot, h] ----
            enc_ps = psa.tile([SLOTS, H], F32, tag="encacc", name="encacc")

            sel_r = sel_d.ap().rearrange("p (c q s) -> p c q s",
                                         c=NCH, q=TT)

            def dma_chunk(ci):
                base, tok = chunks[ci]
                tt = tok // 128
                xt_t = xtp.tile([128, EC, tok], F16, tag="xt", name="xt",
                                padded_shape=[128, EC, TOK])
                nc.sync.dma_start(
                    xt_t[:],
                    xt_d.ap()[:, base:base + tok]
                        .rearrange("(e p) t -> p e t", p=128))
                sel_t = selp.tile([128, tt, SLOTS], F16, tag="sel",
                                  name="sel", padded_shape=[128, TT, SLOTS])
                nc.sync.dma_start(sel_t[:], sel_r[:, ci, 0:tt, :])
                return xt_t, sel_t

            def l1(xt_t, tok):
                h1_t = actp.tile([128, HC, tok], F16, tag="h1", name="h1",
                                 padded_shape=[128, HC, TOK])
                for h in range(HC):
                    ps1 = psp.tile([128, tok], F32, tag="l1", name="l1",
                                   bufs=3, padded_shape=[128, TOK])
                    for e in range(EC):
                        _mm(nc, ps1[:], w1_t[:, e, h, :], xt_t[:, e, :],
                            start=(e == 0), stop=(e == EC - 1))
                    nc.scalar.activation(h1_t[:, h, :], ps1[:],
                                         mybir.ActivationFunctionType.Tanh,
                                         bias=b1_sb[:, h:h + 1])
                return h1_t

            def l2(h1_t, tok):
                tt = tok // 128
                h2_t = actp.tile([128, tt, H], F16, tag="h2", name="h2",
                                 padded_shape=[128, TT, H])
                for t in range(tt):
                    ps2 = psp.tile([128, H], F32, tag="l2", name="l2",
                                   bufs=3)
                    for k in range(HC):
                        _mm(nc, ps2[:], h1_t[:, k, t * 128:(t + 1) * 128],
                            w2_t[:, k, :], start=(k == 0),
                            stop=(k == HC - 1))
                    g2 = actp.tile([128, H], F16, tag="g2", name="g2")
                    nc.vector.tensor_add(g2[:], ps2[:], b2f_sb[:])
                    nc.scalar.activation(h2_t[:, t, :], g2[:],
                                         mybir.ActivationFunctionType.Tanh)
                return h2_t

            def seg(sel_t, h2_t, tok, is_first, is_last):
                tt = tok // 128
                for t in range(tt):
                    _mm(nc, enc_ps[:], sel_t[:, t, :], h2_t[:, t, :],
                        start=(is_first and t == 0),
                        stop=(is_last and t == tt - 1))

            # ---- main loop: PE stream L1(c) | L2(c-1) | seg(c-2) ----
            dma_q = [dma_chunk(ci) for ci in range(min(2, NCH))]
            h1_q = []
            h2_q = []
            for ci in range(NCH):
                if ci + 2 < NCH:
                    dma_q.append(dma_chunk(ci + 2))
                xt_t, sel_t = dma_q[ci]
                h1_q.append((l1(xt_t, chunks[ci][1]), chunks[ci][1]))
                if ci >= 1:
                    h1_t, tok1 = h1_q[ci - 1]
                    h2_q.append((l2(h1_t, tok1), tok1))
                if ci >= 2:
                    h2_t, tok2 = h2_q[ci - 2]
                    seg(dma_q[ci - 2][1], h2_t, tok2,
                        is_first=(ci == 2), is_last=False)
            # epilogue
            h1_t, tok1 = h1_q[NCH - 1]
            h2_q.append((l2(h1_t, tok1), tok1))
            if NCH >= 2:
                seg(dma_q[NCH - 2][1], h2_q[NCH - 2][0], h2_q[NCH - 2][1],
                    is_first=(NCH == 2), is_last=False)
            seg(dma_q[NCH - 1][1], h2_q[NCH - 1][0], h2_q[NCH - 1][1],
                is_first=(NCH == 1), is_last=True)

            # ---- predictor on this core's own <=SLOTS segment rows ----
            # Z = segsum(h2) [SLOTS, H]; q1 = tanh(Z @ WP1 + cnt*b3p1 + pb1)
            z_sb = smp.tile([SLOTS, H], F32, tag="zsb", name="zsb")
            nc.vector.tensor_copy(z_sb[:], enc_ps[:])
            zT = smp.tile([128, HC, SLOTS], F32R, tag="zT", name="zT")
            for k in range(HC):
                pst = psp.tile([128, SLOTS], F32, tag="l1", name="pst",
                               bufs=3)
                nc.tensor.transpose(pst[:], z_sb[:, k * 128:(k + 1) * 128],
                                    ident[:])
                nc.vector.tensor_copy(zT[:, k, :], pst[:])

            q1_sb = smp.tile([128, HC, SLOTS], F32R, tag="q1", name="q1")
            for h in range(HC):
                pp1 = psp.tile([128, SLOTS], F32, tag="l1", name="pp1",
                               bufs=3)
                nc.tensor.matmul(pp1[:], b3p1row[:, h * 128:(h + 1) * 128],
                                 cntrow[:], start=True, stop=False,
                                 skip_group_check=True)
                for k in range(HC):
                    _mm(nc, pp1[:], wp1_t[:, k, h, :], zT[:, k, :],
                        start=False, stop=(k == HC - 1))
                nc.scalar.activation(q1_sb[:, h, :], pp1[:],
                                     mybir.ActivationFunctionType.Tanh,
                                     bias=pb1_sb[:, h:h + 1])
            q2_sb = smp.tile([128, HC, SLOTS], F32R, tag="q2", name="q2")
            for h in range(HC):
                pp2 = psp.tile([128, SLOTS], F32, tag="l1", name="pp2",
                               bufs=3)
                for k in range(HC):
                    _mm(nc, pp2[:], p2_t[:, k, h, :], q1_sb[:, k, :],
                        start=(k == 0), stop=(k == HC - 1))
                nc.scalar.activation(q2_sb[:, h, :], pp2[:],
                                     mybir.ActivationFunctionType.Tanh,
                                     bias=pb2_sb[:, h:h + 1])

            # final: pred[slot, o] = q2.T @ P3 + pb3
            ppo = psp.tile([SLOTS, O], F32, tag="l2", name="ppo", bufs=3)
            nc.tensor.matmul(ppo[:], ones1[:], pb3row[:],
                             start=True, stop=False, skip_group_check=True)
            for k in range(HC):
                _mm(nc, ppo[:], q2_sb[:, k, :], p3_t[:, k, :],
                    start=False, stop=(k == HC - 1))
            pred_sb = smp.tile([SLOTS, O], F32, tag="pred", name="predsb")
            nc.vector.tensor_copy(pred_sb[:], ppo[:])
            nc.sync.dma_start(out_d.ap(), pred_sb[:])

    nc.compile()
    return nc


def kernel(words, seg_ids, W1, b1, W2, b2, W3, b3,
           P1, pb1, P2, pb2, P3, pb3, batch_size, alpha_iter, **_):
    words = np.asarray(words, dtype=np.float32)
    seg_ids = np.asarray(seg_ids).astype(np.int64)
    assert words.shape == (T, E), words.shape
    bs, ai = int(batch_size), int(alpha_iter)

    # --- host-side index prep: cut the sorted token axis at segment
    # boundaries so each core owns whole segments ---
    counts = np.bincount(seg_ids, minlength=S)[:S]
    starts = np.concatenate([[0], np.cumsum(counts)])   # [S+1]
    cuts = [0]
    for c in range(1, N_CORES):
        tgt = c * T // N_CORES
        j = int(np.searchsorted(starts, tgt, side="left"))
        if j > 0 and tgt - starts[j - 1] < starts[j] - tgt:
            j -= 1
        cuts.append(int(starts[j]))
    cuts.append(T)
    lens = np.diff(cuts)
    t_sh = int(np.ceil(lens.max() / 128) * 128)

    # contiguous segment range owned by each core
    seg_lo = [0] * N_CORES
    for c in range(N_CORES - 1, 0, -1):
        if lens[c] > 0:
            seg_lo[c] = int(seg_ids[cuts[c]])
        else:
            seg_lo[c] = S if c == N_CORES - 1 else seg_lo[c + 1]
    seg_hi = seg_lo[1:] + [S]
    slots_needed = max(seg_hi[c] - seg_lo[c] for c in range(N_CORES))
    SLOTS = min(128, max(MIN_SLOTS, ((slots_needed + 31) // 32) * 32))
    assert slots_needed <= SLOTS, (seg_lo, seg_hi)
    assert bs * ai == S

    xt = np.ascontiguousarray(words.T.astype(np.float16))    # [E, T] fp16
    n_full = t_sh // TOK
    tail = t_sh - n_full * TOK
    NCH = n_full + (1 if tail else 0)

    key = ("nc", t_sh, SLOTS)
    if key not in _CACHE:
        _CACHE[key] = _build_nc(t_sh, SLOTS)
    nc = _CACHE[key]

    # --- host-side weight pre-shuffles (dense [128, X] blocks) ---
    W1f, W2f = np.float32(W1), np.float32(W2)
    WP1 = (np.float32(W3) @ np.float32(P1)).astype(np.float32)  # [H, H]
    b3p1 = (np.float32(b3) @ np.float32(P1)).astype(np.float32)  # [H]

    def kmaj_tiles(Wm, dtype):
        # W [K, N] -> [128, K//128, N//128, 128] p-major
        K, N = Wm.shape
        return np.ascontiguousarray(
            Wm.reshape(K // 128, 128, N // 128, 128)
              .transpose(1, 0, 2, 3).reshape(128, -1).astype(dtype))

    common = {
        "w1": kmaj_tiles(W1f.astype(np.float16), np.float16),
        "w2": np.ascontiguousarray(
            W2f.astype(np.float16).reshape(HC, 128, H)
               .transpose(1, 0, 2).reshape(128, -1)),
        "b1": np.ascontiguousarray(
            np.float32(b1).reshape(HC, 128).T),
        "b2f": np.ascontiguousarray(
            np.broadcast_to(np.float32(b2), (128, H))),
        "wp1": kmaj_tiles(WP1, np.float32),
        "b3p1": b3p1.reshape(1, H),
        "p2": kmaj_tiles(np.float32(P2), np.float32),
        "p3": np.ascontiguousarray(
            np.float32(P3).reshape(HC, 128, O)
              .transpose(1, 0, 2).reshape(128, -1)),
        "pb1": np.ascontiguousarray(np.float32(pb1).reshape(HC, 128).T),
        "pb2": np.ascontiguousarray(np.float32(pb2).reshape(HC, 128).T),
        "pb3": np.float32(pb3).reshape(1, O),
    }
    in_maps = []
    for c in range(N_CORES):
        lo, hi = cuts[c], cuts[c + 1]
        n = hi - lo
        xt_c = np.zeros((E, t_sh), dtype=np.float16)
        xt_c[:, :n] = xt[:, lo:hi]
        # packed one-hot selector: sel8[p, c, q, s]
        sel_flat = np.zeros((t_sh, SLOTS), dtype=np.float16)
        sel_flat[:n, :] = (seg_ids[lo:hi, None] ==
                           (seg_lo[c] + np.arange(SLOTS))[None, :])
        # token index inside chunk ci, sub-tile q, lane p = ci*TOK + q*128 + p
        sel_c = np.zeros((128, NCH * TT * SLOTS), dtype=np.float16)
        pad = np.zeros((NCH * TOK - t_sh, SLOTS), dtype=np.float16)
        sp = np.concatenate([sel_flat, pad], axis=0) if NCH * TOK > t_sh \
            else sel_flat
        sel_c[:] = (sp.reshape(NCH, TT, 128, SLOTS)
                      .transpose(2, 0, 1, 3).reshape(128, -1))
        cnt_c = np.zeros((1, SLOTS), dtype=np.float32)
        nseg = seg_hi[c] - seg_lo[c]
        cnt_c[0, :nseg] = counts[seg_lo[c]:seg_hi[c]]
        in_maps.append({
            **common,
            "xt": xt_c,
            "sel": sel_c,
            "cnt": cnt_c,
        })

    global _LAST_IN_MAPS
    _LAST_IN_MAPS = in_maps
    res = bass_utils.run_bass_kernel_spmd(nc, in_maps,
                                          core_ids=list(range(N_CORES)))
    pred = np.zeros((S, O), dtype=np.float32)
    for c in range(N_CORES):
        nseg = seg_hi[c] - seg_lo[c]
        if nseg > 0:
            pred[seg_lo[c]:seg_hi[c]] = res.results[c]["pred"][:nseg]
    return pred.reshape(bs, ai, O).astype(np.float32)


_LAST_IN_MAPS = None


# revision 5
# speedup vs baseline: 1.2046x; 1.0086x over previous
"""DeepSetPred Trainium2 kernel: token encoder MLP + segment-sum + predictor
MLP on 8 NeuronCores, zero collectives.

Sharding: the host cuts the (sorted-by-segment) token axis at segment
boundaries, so every segment belongs to exactly one core. Each shard is
padded to a common length with tokens whose one-hot selector row is all
zero. Each core computes the complete segment sums for its own contiguous
range of <=SLOTS segments, runs the predictor on those rows, and writes its
private slice of the output; the host concatenates.

Structure: the encoder's third linear layer commutes with the segment sum
(it sits after the last tanh), so
    segsum(h2 @ W3 + b3) == segsum(h2) @ W3 + counts * b3
and W3 further folds into the predictor's first layer:
    enc @ P1 + pb1 == segsum(h2) @ (W3 @ P1) + counts * (b3 @ P1) + pb1.
The per-token path is only L1 + L2 + a one-hot segsum matmul over h2
(14336 PE rows per 512-token chunk). L2 is computed token-major (h1 tile
stationary, W2 moving) so the segsum needs no transpose; its bias is added
by the DVE from a broadcast tile (ACT bias is per-partition only), then ACT
applies tanh. The PE stream is skewed L1(i) | L2(i-2) | seg(i-3) so neither
the w2 weight DMA at startup nor the DVE+ACT hop ever stalls the PE. All
weights are host-pre-shuffled into dense [128, X] partition-contiguous
blocks; w1/w2 are split across the scalar+vector DMA queues to parallelize
the startup load, and xt uses 2KB partition lines.
"""

import numpy as np

import concourse.mybir as mybir
import concourse.tile as tile
from concourse import bacc
from concourse import bass_utils
from concourse.masks import make_identity

# Problem shapes (hardcoded per contract).
T, E, H, C, O = 131072, 256, 512, 256, 32
S = 128            # num segments
N_CORES = 8
TOK = 512          # tokens per chunk
MIN_SLOTS = 32     # baseline segments-per-core capacity
SG = 4             # chunks per sel DMA group
F32 = mybir.dt.float32
F32R = mybir.dt.float32r
F16 = mybir.dt.float16

EC = E // 128   # 2
HC = H // 128   # 4
TT = TOK // 128  # 4 token sub-tiles per chunk

_CACHE = {}


def _mm(nc, out, lhsT, rhs, start, stop, skip=True):
    nc.tensor.matmul(out, lhsT, rhs,
                     start=start, stop=stop, skip_group_check=skip)


def _build_nc(t_sh, SLOTS):
    assert t_sh % 128 == 0
    n_full = t_sh // TOK
    tail = t_sh - n_full * TOK
    chunks = [(i * TOK, TOK) for i in range(n_full)]
    if tail:
        chunks.append((n_full * TOK, tail))
    NCH = len(chunks)
    NSG = (NCH + SG - 1) // SG

    nc = bacc.Bacc("TRN2", target_bir_lowering=False, debug=False,
                   num_devices=N_CORES)

    # xt packed: [128, NCH, EC, TOK] -> 2KB contiguous per partition/chunk
    xt_d = nc.dram_tensor("xt", [128, NCH * EC * TOK], F16,
                          kind="ExternalInput")
    # sel packed per chunk: [128, NCH, TT, SLOTS] flattened on the free dim
    sel_d = nc.dram_tensor("sel", [128, NCH * TT * SLOTS], F16,
                           kind="ExternalInput")
    cnt_d = nc.dram_tensor("cnt", [1, SLOTS], F32, kind="ExternalInput")
    # dense pre-shuffled weights: [128, ...] partition-major blocks
    w1_d = nc.dram_tensor("w1", [128, HC * EC * 128], F16,
                          kind="ExternalInput")      # h-major tiles
    w2_d = nc.dram_tensor("w2", [128, HC * H], F16, kind="ExternalInput")
    b1_d = nc.dram_tensor("b1", [128, HC], F32, kind="ExternalInput")
    b2f_d = nc.dram_tensor("b2f", [128, H], F32, kind="ExternalInput")
    wp1_d = nc.dram_tensor("wp1", [128, HC * HC * 128], F32,
                           kind="ExternalInput")   # W3 @ P1, k-major tiles
    b3p1_d = nc.dram_tensor("b3p1", [1, H], F32, kind="ExternalInput")
    p2_d = nc.dram_tensor("p2", [128, HC * HC * 128], F32,
                          kind="ExternalInput")
    p3_d = nc.dram_tensor("p3", [128, HC * O], F32, kind="ExternalInput")
    pb1_d = nc.dram_tensor("pb1", [128, HC], F32, kind="ExternalInput")
    pb2_d = nc.dram_tensor("pb2", [128, HC], F32, kind="ExternalInput")
    pb3_d = nc.dram_tensor("pb3", [1, O], F32, kind="ExternalInput")
    out_d = nc.dram_tensor("pred", [SLOTS, O], F32, kind="ExternalOutput")

    with tile.TileContext(nc) as tc:
        with tc.tile_pool(name="wts", bufs=1) as wp, \
             tc.tile_pool(name="xt", bufs=4) as xtp, \
             tc.tile_pool(name="sel", bufs=3) as selp, \
             tc.tile_pool(name="act", bufs=3) as actp, \
             tc.tile_pool(name="small", bufs=1) as smp, \
             tc.tile_pool(name="ps", bufs=2, space="PSUM") as psp, \
             tc.tile_pool(name="psacc", bufs=1, space="PSUM") as psa:

            # warm the ACT tanh table before the queues fill
            warm_sb = smp.tile([1, 1], F32, tag="warm", name="warm")
            nc.gpsimd.memset(warm_sb[:], 0.0)
            warm_o = smp.tile([1, 1], F32, tag="warmo", name="warmo")
            nc.scalar.activation(warm_o[:], warm_sb[:],
                                 mybir.ActivationFunctionType.Tanh)

            # ---- resident weights; every DMA is partition-contiguous.
            # w1/w2 split across the scalar+vector queues so both halves
            # land in parallel while the sync queue streams xt. ----
            w1_t = wp.tile([128, HC, EC, 128], F16, tag="w1", name="w1t")
            w1_r = w1_d.ap().rearrange("p (h e q) -> p h e q", h=HC, e=EC)
            HH = HC // 2
            nc.scalar.dma_start(w1_t[:, 0:HH, :, :], w1_r[:, 0:HH, :, :])
            nc.gpsimd.dma_start(w1_t[:, HH:HC, :, :], w1_r[:, HH:HC, :, :])
            b1_sb = smp.tile([128, HC], F32, tag="b1", name="b1")
            nc.scalar.dma_start(b1_sb[:], b1_d.ap())
            w2_t = wp.tile([128, HC, H], F16, tag="w2", name="w2t")
            w2_r = w2_d.ap().rearrange("p (k j) -> p k j", k=HC)
            nc.scalar.dma_start(w2_t[:, 0:HH, :], w2_r[:, 0:HH, :])
            nc.gpsimd.dma_start(w2_t[:, HH:HC, :], w2_r[:, HH:HC, :])
            b2f_sb = smp.tile([128, H], F32, tag="b2f", name="b2f")
            nc.gpsimd.dma_start(b2f_sb[:], b2f_d.ap())
            wp1_t = wp.tile([128, HC, HC, 128], F32R, tag="wp1", name="wp1t")
            nc.gpsimd.dma_start(
                wp1_t[:], wp1_d.ap().rearrange("p (k h q) -> p k h q",
                                               k=HC, h=HC))
            p2_t = wp.tile([128, HC, HC, 128], F32R, tag="p2", name="p2t")
            nc.gpsimd.dma_start(
                p2_t[:], p2_d.ap().rearrange("p (k h q) -> p k h q",
                                             k=HC, h=HC))
            p3_t = wp.tile([128, HC, O], F32R, tag="p3", name="p3t")
            nc.gpsimd.dma_start(
                p3_t[:], p3_d.ap().rearrange("p (k o) -> p k o", k=HC))
            b3p1row = smp.tile([1, H], F32, tag="b3p1", name="b3p1")
            nc.gpsimd.dma_start(b3p1row[:], b3p1_d.ap())
            pb1_sb = smp.tile([128, HC], F32, tag="pb1", name="pb1")
            nc.gpsimd.dma_start(pb1_sb[:], pb1_d.ap())
            pb2_sb = smp.tile([128, HC], F32, tag="pb2", name="pb2")
            nc.gpsimd.dma_start(pb2_sb[:], pb2_d.ap())
            pb3row = smp.tile([1, O], F32, tag="pb3row", name="pb3row")
            nc.gpsimd.dma_start(pb3row[:], pb3_d.ap())
            cntrow = smp.tile([1, SLOTS], F32, tag="cntrow", name="cntrow")
            nc.gpsimd.dma_start(cntrow[:], cnt_d.ap())
            ones1 = smp.tile([1, SLOTS], F32, tag="ones1", name="ones1")
            nc.gpsimd.memset(ones1[:], 1.0)
            ident = smp.tile([SLOTS, SLOTS], F32, tag="ident", name="ident")
            make_identity(nc, ident[:])

            # ---- persistent segment-sum accumulator Z[slot, h] ----
            enc_ps = psa.tile([SLOTS, H], F32, tag="encacc", name="encacc")

            xt_r = xt_d.ap().rearrange("p (c e t) -> p c e t", c=NCH, e=EC)
            sel_r = sel_d.ap().rearrange("p (c q s) -> p c q s",
                                         c=NCH, q=TT)

            sel_tiles = {}

            def dma_xt(ci):
                tok = chunks[ci][1]
                xt_t = xtp.tile([128, EC, tok], F16, tag="xt", name="xt",
                                padded_shape=[128, EC, TOK])
                nc.sync.dma_start(xt_t[:], xt_r[:, ci, :, 0:tok])
                return xt_t

            def dma_selg(g):
                lo = g * SG
                gsz = min(SG, NCH - lo)
                selg = selp.tile([128, gsz, TT, SLOTS], F16, tag="sel",
                                 name="sel", padded_shape=[128, SG, TT,
                                                           SLOTS])
                nc.sync.dma_start(selg[:], sel_r[:, lo:lo + gsz, :, :])
                sel_tiles[g] = selg

            def l1(xt_t, tok):
                h1_t = actp.tile([128, HC, tok], F16, tag="h1", name="h1",
                                 bufs=4, padded_shape=[128, HC, TOK])
                for h in range(HC):
                    ps1 = psp.tile([128, tok], F32, tag="l1", name="l1",
                                   bufs=3, padded_shape=[128, TOK])
                    for e in range(EC):
                        _mm(nc, ps1[:], w1_t[:, h, e, :], xt_t[:, e, :],
                            start=(e == 0), stop=(e == EC - 1))
                    nc.scalar.activation(h1_t[:, h, :], ps1[:],
                                         mybir.ActivationFunctionType.Tanh,
                                         bias=b1_sb[:, h:h + 1])
                return h1_t

            def l2(h1_t, tok):
                tt = tok // 128
                h2_t = actp.tile([128, tt, H], F16, tag="h2", name="h2",
                                 padded_shape=[128, TT, H])
                for t in range(tt):
                    ps2 = psp.tile([128, H], F32, tag="l2", name="l2",
                                   bufs=3)
                    for k in range(HC):
                        _mm(nc, ps2[:], h1_t[:, k, t * 128:(t + 1) * 128],
                            w2_t[:, k, :], start=(k == 0),
                            stop=(k == HC - 1))
                    g2 = actp.tile([128, H], F16, tag="g2", name="g2")
                    nc.vector.tensor_add(g2[:], ps2[:], b2f_sb[:])
                    nc.scalar.activation(h2_t[:, t, :], g2[:],
                                         mybir.ActivationFunctionType.Tanh)
                return h2_t

            def seg(ci, h2_t, tok, is_first, is_last):
                tt = tok // 128
                selg = sel_tiles[ci // SG]
                for t in range(tt):
                    _mm(nc, enc_ps[:], selg[:, ci % SG, t, :],
                        h2_t[:, t, :],
                        start=(is_first and t == 0),
                        stop=(is_last and t == tt - 1))

            # ---- main loop: PE stream L1(i) | L2(i-2) | seg(i-3) ----
            assert NCH >= 4
            xt_q = [dma_xt(0), dma_xt(1)]
            dma_selg(0)
            h1_q = []
            h2_q = []
            for ci in range(NCH):
                if ci + 2 < NCH:
                    xt_q.append(dma_xt(ci + 2))
                    if (ci + 2) % SG == 0:
                        dma_selg((ci + 2) // SG)
                h1_q.append((l1(xt_q[ci], chunks[ci][1]), chunks[ci][1]))
                if ci >= 2:
                    h1_t, tok1 = h1_q[ci - 2]
                    h2_q.append((l2(h1_t, tok1), tok1))
                if ci >= 3:
                    h2_t, tok2 = h2_q[ci - 3]
                    seg(ci - 3, h2_t, tok2,
                        is_first=(ci == 3), is_last=False)
            # epilogue: remaining L2/seg in dependency-friendly order
            h2_q.append((l2(h1_q[NCH - 2][0], h1_q[NCH - 2][1]),
                         h1_q[NCH - 2][1]))
            seg(NCH - 3, h2_q[NCH - 3][0], h2_q[NCH - 3][1],
                is_first=False, is_last=False)
            h2_q.append((l2(h1_q[NCH - 1][0], h1_q[NCH - 1][1]),
                         h1_q[NCH - 1][1]))
            seg(NCH - 2, h2_q[NCH - 2][0], h2_q[NCH - 2][1],
                is_first=False, is_last=False)
            seg(NCH - 1, h2_q[NCH - 1][0], h2_q[NCH - 1][1],
                is_first=False, is_last=True)

            # ---- predictor on this core's own <=SLOTS segment rows ----
            # Z = segsum(h2) [SLOTS, H]; q1 = tanh(Z @ WP1 + cnt*b3p1 + pb1)
            z_sb = smp.tile([SLOTS, H], F32, tag="zsb", name="zsb")
            nc.vector.tensor_copy(z_sb[:], enc_ps[:])
            zT = smp.tile([128, HC, SLOTS], F32R, tag="zT", name="zT")
            for k in range(HC):
                pst = psp.tile([128, SLOTS], F32, tag="l1", name="pst",
                               bufs=3)
                nc.tensor.transpose(pst[:], z_sb[:, k * 128:(k + 1) * 128],
                                    ident[:])
                nc.vector.tensor_copy(zT[:, k, :], pst[:])

            q1_sb = smp.tile([128, HC, SLOTS], F32R, tag="q1", name="q1")
            for h in range(HC):
                pp1 = psp.tile([128, SLOTS], F32, tag="l1", name="pp1",
                               bufs=3)
                nc.tensor.matmul(pp1[:], b3p1row[:, h * 128:(h + 1) * 128],
                                 cntrow[:], start=True, stop=False,
                                 skip_group_check=True)
                for k in range(HC):
                    _mm(nc, pp1[:], wp1_t[:, k, h, :], zT[:, k, :],
                        start=False, stop=(k == HC - 1))
                nc.scalar.activation(q1_sb[:, h, :], pp1[:],
                                     mybir.ActivationFunctionType.Tanh,
                                     bias=pb1_sb[:, h:h + 1])
            q2_sb = smp.tile([128, HC, SLOTS], F32R, tag="q2", name="q2")
            for h in range(HC):
                pp2 = psp.tile([128, SLOTS], F32, tag="l1", name="pp2",
                               bufs=3)
                for k in range(HC):
                    _mm(nc, pp2[:], p2_t[:, k, h, :], q1_sb[:, k, :],
                        start=(k == 0), stop=(k == HC - 1))
                nc.scalar.activation(q2_sb[:, h, :], pp2[:],
                                     mybir.ActivationFunctionType.Tanh,
                                     bias=pb2_sb[:, h:h + 1])

            # final: pred[slot, o] = q2.T @ P3 + pb3
            ppo = psp.tile([SLOTS, O], F32, tag="l2", name="ppo", bufs=3)
            nc.tensor.matmul(ppo[:], ones1[:], pb3row[:],
                             start=True, stop=False, skip_group_check=True)
            for k in range(HC):
                _mm(nc, ppo[:], q2_sb[:, k, :], p3_t[:, k, :],
                    start=False, stop=(k == HC - 1))
            pred_sb = smp.tile([SLOTS, O], F32, tag="pred", name="predsb")
            nc.vector.tensor_copy(pred_sb[:], ppo[:])
            nc.sync.dma_start(out_d.ap(), pred_sb[:])

    nc.compile()
    return nc


def kernel(words, seg_ids, W1, b1, W2, b2, W3, b3,
           P1, pb1, P2, pb2, P3, pb3, batch_size, alpha_iter, **_):
    words = np.asarray(words, dtype=np.float32)
    seg_ids = np.asarray(seg_ids).astype(np.int64)
    assert words.shape == (T, E), words.shape
    bs, ai = int(batch_size), int(alpha_iter)

    # --- host-side index prep: cut the sorted token axis at segment
    # boundaries so each core owns whole segments ---
    counts = np.bincount(seg_ids, minlength=S)[:S]
    starts = np.concatenate([[0], np.cumsum(counts)])   # [S+1]
    cuts = [0]
    for c in range(1, N_CORES):
        tgt = c * T // N_CORES
        j = int(np.searchsorted(starts, tgt, side="left"))
        if j > 0 and tgt - starts[j - 1] < starts[j] - tgt:
            j -= 1
        cuts.append(int(starts[j]))
    cuts.append(T)
    lens = np.diff(cuts)
    t_sh = int(np.ceil(lens.max() / 128) * 128)

    # contiguous segment range owned by each core
    seg_lo = [0] * N_CORES
    for c in range(N_CORES - 1, 0, -1):
        if lens[c] > 0:
            seg_lo[c] = int(seg_ids[cuts[c]])
        else:
            seg_lo[c] = S if c == N_CORES - 1 else seg_lo[c + 1]
    seg_hi = seg_lo[1:] + [S]
    slots_needed = max(seg_hi[c] - seg_lo[c] for c in range(N_CORES))
    SLOTS = min(128, max(MIN_SLOTS, ((slots_needed + 31) // 32) * 32))
    assert slots_needed <= SLOTS, (seg_lo, seg_hi)
    assert bs * ai == S

    xt = np.ascontiguousarray(words.T.astype(np.float16))    # [E, T] fp16
    n_full = t_sh // TOK
    tail = t_sh - n_full * TOK
    NCH = n_full + (1 if tail else 0)

    key = ("nc", t_sh, SLOTS)
    if key not in _CACHE:
        _CACHE[key] = _build_nc(t_sh, SLOTS)
    nc = _CACHE[key]

    # --- host-side weight pre-shuffles (dense [128, X] blocks) ---
    W1f, W2f = np.float32(W1), np.float32(W2)
    WP1 = (np.float32(W3) @ np.float32(P1)).astype(np.float32)  # [H, H]
    b3p1 = (np.float32(b3) @ np.float32(P1)).astype(np.float32)  # [H]

    def kmaj_tiles(Wm, dtype):
        # W [K, N] -> [128, K//128, N//128, 128] p-major
        K, N = Wm.shape
        return np.ascontiguousarray(
            Wm.reshape(K // 128, 128, N // 128, 128)
              .transpose(1, 0, 2, 3).reshape(128, -1).astype(dtype))

    common = {
        # w1 h-major: [128, HC, EC, 128]
        "w1": np.ascontiguousarray(
            W1f.astype(np.float16).reshape(EC, 128, HC, 128)
               .transpose(1, 2, 0, 3).reshape(128, -1)),
        "w2": np.ascontiguousarray(
            W2f.astype(np.float16).reshape(HC, 128, H)
               .transpose(1, 0, 2).reshape(128, -1)),
        "b1": np.ascontiguousarray(
            np.float32(b1).reshape(HC, 128).T),
        "b2f": np.ascontiguousarray(
            np.broadcast_to(np.float32(b2), (128, H))),
        "wp1": kmaj_tiles(WP1, np.float32),
        "b3p1": b3p1.reshape(1, H),
        "p2": kmaj_tiles(np.float32(P2), np.float32),
        "p3": np.ascontiguousarray(
            np.float32(P3).reshape(HC, 128, O)
              .transpose(1, 0, 2).reshape(128, -1)),
        "pb1": np.ascontiguousarray(np.float32(pb1).reshape(HC, 128).T),
        "pb2": np.ascontiguousarray(np.float32(pb2).reshape(HC, 128).T),
        "pb3": np.float32(pb3).reshape(1, O),
    }
    in_maps = []
    for c in range(N_CORES):
        lo, hi = cuts[c], cuts[c + 1]
        n = hi - lo
        # xt packed [128, NCH, EC, TOK]: 2KB lines per partition/chunk
        xt_flat = np.zeros((E, NCH * TOK), dtype=np.float16)
        xt_flat[:, :n] = xt[:, lo:hi]
        xt_c = np.ascontiguousarray(
            xt_flat.reshape(EC, 128, NCH, TOK)
                   .transpose(1, 2, 0, 3).reshape(128, -1))
        # packed one-hot selector: sel8[p, ci, q, s]
        sel_flat = np.zeros((NCH * TOK, SLOTS), dtype=np.float16)
        sel_flat[:n, :] = (seg_ids[lo:hi, None] ==
                           (seg_lo[c] + np.arange(SLOTS))[None, :])
        sel_c = np.ascontiguousarray(
            sel_flat.reshape(NCH, TT, 128, SLOTS)
                    .transpose(2, 0, 1, 3).reshape(128, -1))
        cnt_c = np.zeros((1, SLOTS), dtype=np.float32)
        nseg = seg_hi[c] - seg_lo[c]
        cnt_c[0, :nseg] = counts[seg_lo[c]:seg_hi[c]]
        in_maps.append({
            **common,
            "xt": xt_c,
            "sel": sel_c,
            "cnt": cnt_c,
        })

    global _LAST_IN_MAPS
    _LAST_IN_MAPS = in_maps
    res = bass_utils.run_bass_kernel_spmd(nc, in_maps,
                                          core_ids=list(range(N_CORES)))
    pred = np.zeros((S, O), dtype=np.float32)
    for c in range(N_CORES):
        nseg = seg_hi[c] - seg_lo[c]
        if nseg > 0:
            pred[seg_lo[c]:seg_hi[c]] = res.results[c]["pred"][:nseg]
    return pred.reshape(bs, ai, O).astype(np.float32)


_LAST_IN_MAPS = None


# revision 6
# speedup vs baseline: 1.2199x; 1.0127x over previous
"""DeepSetPred Trainium2 kernel: token encoder MLP + segment-sum + predictor
MLP on 8 NeuronCores, zero collectives.

Sharding: the host cuts the (sorted-by-segment) token axis at segment
boundaries, so every segment belongs to exactly one core. Each shard is
padded to a common length with tokens whose one-hot selector row is all
zero. Each core computes the complete segment sums for its own contiguous
range of <=SLOTS segments, runs the predictor on those rows, and writes its
private slice of the output; the host concatenates.

Structure: the encoder's third linear layer commutes with the segment sum
(it sits after the last tanh), so
    segsum(h2 @ W3 + b3) == segsum(h2) @ W3 + counts * b3
and W3 further folds into the predictor's first layer:
    enc @ P1 + pb1 == segsum(h2) @ (W3 @ P1) + counts * (b3 @ P1) + pb1.
The per-token path is only L1 + L2 + a one-hot segsum matmul over h2
(14336 PE rows per 512-token chunk). L2 is computed token-major (h1 tile
stationary, W2 moving) so the segsum needs no transpose; its bias is added
by the DVE from a broadcast tile (ACT bias is per-partition only), then ACT
applies tanh. The PE stream is skewed L1(i) | L2(i-2) | seg(i-3) so neither
the w2 weight DMA at startup nor the DVE+ACT hop ever stalls the PE. All
weights are host-pre-shuffled into dense [128, X] partition-contiguous
blocks; w1/w2 are split across the scalar+vector DMA queues to parallelize
the startup load, and xt uses 2KB partition lines.
"""

import numpy as np

import concourse.mybir as mybir
import concourse.tile as tile
from concourse import bacc
from concourse import bass_utils
from concourse.masks import make_identity

# Problem shapes (hardcoded per contract).
T, E, H, C, O = 131072, 256, 512, 256, 32
S = 128            # num segments
N_CORES = 8
TOK = 512          # tokens per chunk
MIN_SLOTS = 32     # baseline segments-per-core capacity
SG = 4             # chunks per sel DMA group
F32 = mybir.dt.float32
F32R = mybir.dt.float32r
F16 = mybir.dt.float16

EC = E // 128   # 2
HC = H // 128   # 4
TT = TOK // 128  # 4 token sub-tiles per chunk

_CACHE = {}


def _mm(nc, out, lhsT, rhs, start, stop, skip=True):
    nc.tensor.matmul(out, lhsT, rhs,
                     start=start, stop=stop, skip_group_check=skip)


def _build_nc(t_sh, SLOTS):
    assert t_sh % 128 == 0
    n_full = t_sh // TOK
    tail = t_sh - n_full * TOK
    chunks = [(i * TOK, TOK) for i in range(n_full)]
    if tail:
        chunks.append((n_full * TOK, tail))
    NCH = len(chunks)
    NSG = (NCH + SG - 1) // SG

    nc = bacc.Bacc("TRN2", target_bir_lowering=False, debug=False,
                   num_devices=N_CORES)

    # xt packed: [128, NCH, EC, TOK] -> 2KB contiguous per partition/chunk
    xt_d = nc.dram_tensor("xt", [128, NCH * EC * TOK], F16,
                          kind="ExternalInput")
    # sel packed per chunk: [128, NCH, TT, SLOTS] flattened on the free dim
    sel_d = nc.dram_tensor("sel", [128, NCH * TT * SLOTS], F16,
                           kind="ExternalInput")
    cnt_d = nc.dram_tensor("cnt", [1, SLOTS], F32, kind="ExternalInput")
    # dense pre-shuffled weights: [128, ...] partition-major blocks
    w1_d = nc.dram_tensor("w1", [128, HC * EC * 128], F16,
                          kind="ExternalInput")      # h-major tiles
    w2_d = nc.dram_tensor("w2", [128, HC * H], F16, kind="ExternalInput")
    b1_d = nc.dram_tensor("b1", [128, HC], F32, kind="ExternalInput")
    b2f_d = nc.dram_tensor("b2f", [128, H], F32, kind="ExternalInput")
    wp1_d = nc.dram_tensor("wp1", [128, HC * HC * 128], F32,
                           kind="ExternalInput")   # W3 @ P1, k-major tiles
    b3p1_d = nc.dram_tensor("b3p1", [1, H], F32, kind="ExternalInput")
    p2_d = nc.dram_tensor("p2", [128, HC * HC * 128], F32,
                          kind="ExternalInput")
    p3_d = nc.dram_tensor("p3", [128, HC * O], F32, kind="ExternalInput")
    pb1_d = nc.dram_tensor("pb1", [128, HC], F32, kind="ExternalInput")
    pb2_d = nc.dram_tensor("pb2", [128, HC], F32, kind="ExternalInput")
    pb3_d = nc.dram_tensor("pb3", [1, O], F32, kind="ExternalInput")
    out_d = nc.dram_tensor("pred", [SLOTS, O], F32, kind="ExternalOutput")

    with tile.TileContext(nc) as tc:
        with tc.tile_pool(name="wts", bufs=1) as wp, \
             tc.tile_pool(name="xt", bufs=4) as xtp, \
             tc.tile_pool(name="sel", bufs=3) as selp, \
             tc.tile_pool(name="act", bufs=3) as actp, \
             tc.tile_pool(name="small", bufs=1) as smp, \
             tc.tile_pool(name="ps", bufs=2, space="PSUM") as psp, \
             tc.tile_pool(name="psacc", bufs=1, space="PSUM") as psa:

            # warm the ACT tanh table before the queues fill
            warm_sb = smp.tile([1, 1], F32, tag="warm", name="warm")
            nc.gpsimd.memset(warm_sb[:], 0.0)
            warm_o = smp.tile([1, 1], F32, tag="warmo", name="warmo")
            nc.scalar.activation(warm_o[:], warm_sb[:],
                                 mybir.ActivationFunctionType.Tanh)

            # ---- resident weights; every DMA is partition-contiguous.
            # w1/w2 split across the scalar+vector queues so both halves
            # land in parallel while the sync queue streams xt. ----
            # w1 (whole, 2KB lines) + b1 on the scalar ring; w2 (whole,
            # 4KB lines) leads the gpsimd ring so L2(0) is never blocked.
            w1_t = wp.tile([128, HC, EC, 128], F16, tag="w1", name="w1t")
            w1_r = w1_d.ap().rearrange("p (h e q) -> p h e q", h=HC, e=EC)
            nc.scalar.dma_start(w1_t[:], w1_r)
            b1_sb = smp.tile([128, HC], F32, tag="b1", name="b1")
            nc.scalar.dma_start(b1_sb[:], b1_d.ap())
            w2_t = wp.tile([128, HC, H], F16, tag="w2", name="w2t")
            w2_r = w2_d.ap().rearrange("p (k j) -> p k j", k=HC)
            nc.gpsimd.dma_start(w2_t[:], w2_r)
            b2f_sb = smp.tile([128, H], F32, tag="b2f", name="b2f")
            nc.gpsimd.dma_start(b2f_sb[:], b2f_d.ap())
            wp1_t = wp.tile([128, HC, HC, 128], F32R, tag="wp1", name="wp1t")
            nc.gpsimd.dma_start(
                wp1_t[:], wp1_d.ap().rearrange("p (k h q) -> p k h q",
                                               k=HC, h=HC))
            p2_t = wp.tile([128, HC, HC, 128], F32R, tag="p2", name="p2t")
            nc.gpsimd.dma_start(
                p2_t[:], p2_d.ap().rearrange("p (k h q) -> p k h q",
                                             k=HC, h=HC))
            p3_t = wp.tile([128, HC, O], F32R, tag="p3", name="p3t")
            nc.gpsimd.dma_start(
                p3_t[:], p3_d.ap().rearrange("p (k o) -> p k o", k=HC))
            b3p1row = smp.tile([1, H], F32, tag="b3p1", name="b3p1")
            nc.gpsimd.dma_start(b3p1row[:], b3p1_d.ap())
            pb1_sb = smp.tile([128, HC], F32, tag="pb1", name="pb1")
            nc.gpsimd.dma_start(pb1_sb[:], pb1_d.ap())
            pb2_sb = smp.tile([128, HC], F32, tag="pb2", name="pb2")
            nc.gpsimd.dma_start(pb2_sb[:], pb2_d.ap())
            pb3row = smp.tile([1, O], F32, tag="pb3row", name="pb3row")
            nc.gpsimd.dma_start(pb3row[:], pb3_d.ap())
            cntrow = smp.tile([1, SLOTS], F32, tag="cntrow", name="cntrow")
            nc.gpsimd.dma_start(cntrow[:], cnt_d.ap())
            ones1 = smp.tile([1, SLOTS], F32, tag="ones1", name="ones1")
            nc.gpsimd.memset(ones1[:], 1.0)
            ident = smp.tile([SLOTS, SLOTS], F32, tag="ident", name="ident")
            make_identity(nc, ident[:])

            # ---- persistent segment-sum accumulator Z[slot, h] ----
            enc_ps = psa.tile([SLOTS, H], F32, tag="encacc", name="encacc")

            xt_r = xt_d.ap().rearrange("p (c e t) -> p c e t", c=NCH, e=EC)
            sel_r = sel_d.ap().rearrange("p (c q s) -> p c q s",
                                         c=NCH, q=TT)

            sel_tiles = {}

            def dma_xt(ci):
                tok = chunks[ci][1]
                xt_t = xtp.tile([128, EC, tok], F16, tag="xt", name="xt",
                                padded_shape=[128, EC, TOK])
                nc.sync.dma_start(xt_t[:], xt_r[:, ci, :, 0:tok])
                return xt_t

            def dma_selg(g):
                lo = g * SG
                gsz = min(SG, NCH - lo)
                selg = selp.tile([128, gsz, TT, SLOTS], F16, tag="sel",
                                 name="sel", padded_shape=[128, SG, TT,
                                                           SLOTS])
                nc.sync.dma_start(selg[:], sel_r[:, lo:lo + gsz, :, :])
                sel_tiles[g] = selg

            def l1(xt_t, tok):
                h1_t = actp.tile([128, HC, tok], F16, tag="h1", name="h1",
                                 bufs=4, padded_shape=[128, HC, TOK])
                for h in range(HC):
                    ps1 = psp.tile([128, tok], F32, tag="l1", name="l1",
                                   bufs=3, padded_shape=[128, TOK])
                    for e in range(EC):
                        _mm(nc, ps1[:], w1_t[:, h, e, :], xt_t[:, e, :],
                            start=(e == 0), stop=(e == EC - 1))
                    nc.scalar.activation(h1_t[:, h, :], ps1[:],
                                         mybir.ActivationFunctionType.Tanh,
                                         bias=b1_sb[:, h:h + 1])
                return h1_t

            def l2(h1_t, tok):
                tt = tok // 128
                h2_t = actp.tile([128, tt, H], F16, tag="h2", name="h2",
                                 padded_shape=[128, TT, H])
                for t in range(tt):
                    ps2 = psp.tile([128, H], F32, tag="l2", name="l2",
                                   bufs=3)
                    for k in range(HC):
                        _mm(nc, ps2[:], h1_t[:, k, t * 128:(t + 1) * 128],
                            w2_t[:, k, :], start=(k == 0),
                            stop=(k == HC - 1))
                    g2 = actp.tile([128, H], F16, tag="g2", name="g2")
                    nc.vector.tensor_add(g2[:], ps2[:], b2f_sb[:])
                    nc.scalar.activation(h2_t[:, t, :], g2[:],
                                         mybir.ActivationFunctionType.Tanh)
                return h2_t

            def seg(ci, h2_t, tok, is_first, is_last):
                tt = tok // 128
                selg = sel_tiles[ci // SG]
                for t in range(tt):
                    _mm(nc, enc_ps[:], selg[:, ci % SG, t, :],
                        h2_t[:, t, :],
                        start=(is_first and t == 0),
                        stop=(is_last and t == tt - 1))

            # ---- main loop: PE stream L1(i) | L2(i-2) | seg(i-3) ----
            assert NCH >= 4
            xt_q = [dma_xt(0), dma_xt(1)]
            dma_selg(0)
            h1_q = []
            h2_q = []
            for ci in range(NCH):
                if ci + 2 < NCH:
                    xt_q.append(dma_xt(ci + 2))
                    if (ci + 2) % SG == 0:
                        dma_selg((ci + 2) // SG)
                h1_q.append((l1(xt_q[ci], chunks[ci][1]), chunks[ci][1]))
                if ci >= 2:
                    h1_t, tok1 = h1_q[ci - 2]
                    h2_q.append((l2(h1_t, tok1), tok1))
                if ci >= 3:
                    h2_t, tok2 = h2_q[ci - 3]
                    seg(ci - 3, h2_t, tok2,
                        is_first=(ci == 3), is_last=False)
            # epilogue: remaining L2/seg in dependency-friendly order
            h2_q.append((l2(h1_q[NCH - 2][0], h1_q[NCH - 2][1]),
                         h1_q[NCH - 2][1]))
            seg(NCH - 3, h2_q[NCH - 3][0], h2_q[NCH - 3][1],
                is_first=False, is_last=False)
            h2_q.append((l2(h1_q[NCH - 1][0], h1_q[NCH - 1][1]),
                         h1_q[NCH - 1][1]))
            seg(NCH - 2, h2_q[NCH - 2][0], h2_q[NCH - 2][1],
                is_first=False, is_last=False)
            seg(NCH - 1, h2_q[NCH - 1][0], h2_q[NCH - 1][1],
                is_first=False, is_last=True)

            # ---- predictor on this core's own <=SLOTS segment rows ----
            # Z = segsum(h2) [SLOTS, H]; q1 = tanh(Z @ WP1 + cnt*b3p1 + pb1)
            # slice-pipelined: copy k-slice, transpose it while the next
            # slice copies, so the chain latency overlaps
            z_sb = smp.tile([SLOTS, H], F32, tag="zsb", name="zsb")
            zT = smp.tile([128, HC, SLOTS], F32R, tag="zT", name="zT")
            for k in range(HC):
                nc.vector.tensor_copy(z_sb[:, k * 128:(k + 1) * 128],
                                      enc_ps[:, k * 128:(k + 1) * 128])
            for k in range(HC):
                pst = psp.tile([128, SLOTS], F32, tag="l1", name="pst",
                               bufs=3)
                nc.tensor.transpose(pst[:], z_sb[:, k * 128:(k + 1) * 128],
                                    ident[:])
                nc.vector.tensor_copy(zT[:, k, :], pst[:])

            q1_sb = smp.tile([128, HC, SLOTS], F32R, tag="q1", name="q1")
            for h in range(HC):
                pp1 = psp.tile([128, SLOTS], F32, tag="l1", name="pp1",
                               bufs=3)
                nc.tensor.matmul(pp1[:], b3p1row[:, h * 128:(h + 1) * 128],
                                 cntrow[:], start=True, stop=False,
                                 skip_group_check=True)
                for k in range(HC):
                    _mm(nc, pp1[:], wp1_t[:, k, h, :], zT[:, k, :],
                        start=False, stop=(k == HC - 1))
                nc.scalar.activation(q1_sb[:, h, :], pp1[:],
                                     mybir.ActivationFunctionType.Tanh,
                                     bias=pb1_sb[:, h:h + 1])
            q2_sb = smp.tile([128, HC, SLOTS], F32R, tag="q2", name="q2")
            for h in range(HC):
                pp2 = psp.tile([128, SLOTS], F32, tag="l1", name="pp2",
                               bufs=3)
                for k in range(HC):
                    _mm(nc, pp2[:], p2_t[:, k, h, :], q1_sb[:, k, :],
                        start=(k == 0), stop=(k == HC - 1))
                nc.scalar.activation(q2_sb[:, h, :], pp2[:],
                                     mybir.ActivationFunctionType.Tanh,
                                     bias=pb2_sb[:, h:h + 1])

            # final: pred[slot, o] = q2.T @ P3 + pb3
            ppo = psp.tile([SLOTS, O], F32, tag="l2", name="ppo", bufs=3)
            nc.tensor.matmul(ppo[:], ones1[:], pb3row[:],
                             start=True, stop=False, skip_group_check=True)
            for k in range(HC):
                _mm(nc, ppo[:], q2_sb[:, k, :], p3_t[:, k, :],
                    start=False, stop=(k == HC - 1))
            pred_sb = smp.tile([SLOTS, O], F32, tag="pred", name="predsb")
            nc.vector.tensor_copy(pred_sb[:], ppo[:])
            nc.sync.dma_start(out_d.ap(), pred_sb[:])

    nc.compile()
    return nc


def kernel(words, seg_ids, W1, b1, W2, b2, W3, b3,
           P1, pb1, P2, pb2, P3, pb3, batch_size, alpha_iter, **_):
    words = np.asarray(words, dtype=np.float32)
    seg_ids = np.asarray(seg_ids).astype(np.int64)
    assert words.shape == (T, E), words.shape
    bs, ai = int(batch_size), int(alpha_iter)

    # --- host-side index prep: cut the sorted token axis at segment
    # boundaries so each core owns whole segments ---
    counts = np.bincount(seg_ids, minlength=S)[:S]
    starts = np.concatenate([[0], np.cumsum(counts)])   # [S+1]
    cuts = [0]
    for c in range(1, N_CORES):
        tgt = c * T // N_CORES
        j = int(np.searchsorted(starts, tgt, side="left"))
        if j > 0 and tgt - starts[j - 1] < starts[j] - tgt:
            j -= 1
        cuts.append(int(starts[j]))
    cuts.append(T)
    lens = np.diff(cuts)
    t_sh = int(np.ceil(lens.max() / 128) * 128)

    # contiguous segment range owned by each core
    seg_lo = [0] * N_CORES
    for c in range(N_CORES - 1, 0, -1):
        if lens[c] > 0:
            seg_lo[c] = int(seg_ids[cuts[c]])
        else:
            seg_lo[c] = S if c == N_CORES - 1 else seg_lo[c + 1]
    seg_hi = seg_lo[1:] + [S]
    slots_needed = max(seg_hi[c] - seg_lo[c] for c in range(N_CORES))
    SLOTS = min(128, max(MIN_SLOTS, ((slots_needed + 31) // 32) * 32))
    assert slots_needed <= SLOTS, (seg_lo, seg_hi)
    assert bs * ai == S

    xt = np.ascontiguousarray(words.T.astype(np.float16))    # [E, T] fp16
    n_full = t_sh // TOK
    tail = t_sh - n_full * TOK
    NCH = n_full + (1 if tail else 0)

    key = ("nc", t_sh, SLOTS)
    if key not in _CACHE:
        _CACHE[key] = _build_nc(t_sh, SLOTS)
    nc = _CACHE[key]

    # --- host-side weight pre-shuffles (dense [128, X] blocks) ---
    W1f, W2f = np.float32(W1), np.float32(W2)
    WP1 = (np.float32(W3) @ np.float32(P1)).astype(np.float32)  # [H, H]
    b3p1 = (np.float32(b3) @ np.float32(P1)).astype(np.float32)  # [H]

    def kmaj_tiles(Wm, dtype):
        # W [K, N] -> [128, K//128, N//128, 128] p-major
        K, N = Wm.shape
        return np.ascontiguousarray(
            Wm.reshape(K // 128, 128, N // 128, 128)
              .transpose(1, 0, 2, 3).reshape(128, -1).astype(dtype))

    common = {
        # w1 h-major: [128, HC, EC, 128]
        "w1": np.ascontiguousarray(
            W1f.astype(np.float16).reshape(EC, 128, HC, 128)
               .transpose(1, 2, 0, 3).reshape(128, -1)),
        "w2": np.ascontiguousarray(
            W2f.astype(np.float16).reshape(HC, 128, H)
               .transpose(1, 0, 2).reshape(128, -1)),
        "b1": np.ascontiguousarray(
            np.float32(b1).reshape(HC, 128).T),
        "b2f": np.ascontiguousarray(
            np.broadcast_to(np.float32(b2), (128, H))),
        "wp1": kmaj_tiles(WP1, np.float32),
        "b3p1": b3p1.reshape(1, H),
        "p2": kmaj_tiles(np.float32(P2), np.float32),
        "p3": np.ascontiguousarray(
            np.float32(P3).reshape(HC, 128, O)
              .transpose(1, 0, 2).reshape(128, -1)),
        "pb1": np.ascontiguousarray(np.float32(pb1).reshape(HC, 128).T),
        "pb2": np.ascontiguousarray(np.float32(pb2).reshape(HC, 128).T),
        "pb3": np.float32(pb3).reshape(1, O),
    }
    in_maps = []
    for c in range(N_CORES):
        lo, hi = cuts[c], cuts[c + 1]
        n = hi - lo
        # xt packed [128, NCH, EC, TOK]: 2KB lines per partition/chunk
        xt_flat = np.zeros((E, NCH * TOK), dtype=np.float16)
        xt_flat[:, :n] = xt[:, lo:hi]
        xt_c = np.ascontiguousarray(
            xt_flat.reshape(EC, 128, NCH, TOK)
                   .transpose(1, 2, 0, 3).reshape(128, -1))
        # packed one-hot selector: sel8[p, ci, q, s]
        sel_flat = np.zeros((NCH * TOK, SLOTS), dtype=np.float16)
        sel_flat[:n, :] = (seg_ids[lo:hi, None] ==
                           (seg_lo[c] + np.arange(SLOTS))[None, :])
        sel_c = np.ascontiguousarray(
            sel_flat.reshape(NCH, TT, 128, SLOTS)
                    .transpose(2, 0, 1, 3).reshape(128, -1))
        cnt_c = np.zeros((1, SLOTS), dtype=np.float32)
        nseg = seg_hi[c] - seg_lo[c]
        cnt_c[0, :nseg] = counts[seg_lo[c]:seg_hi[c]]
        in_maps.append({
            **common,
            "xt": xt_c,
            "sel": sel_c,
            "cnt": cnt_c,
        })

    global _LAST_IN_MAPS
    _LAST_IN_MAPS = in_maps
    res = bass_utils.run_bass_kernel_spmd(nc, in_maps,
                                          core_ids=list(range(N_CORES)))
    pred = np.zeros((S, O), dtype=np.float32)
    for c in range(N_CORES):
        nseg = seg_hi[c] - seg_lo[c]
        if nseg > 0:
            pred[seg_lo[c]:seg_hi[c]] = res.results[c]["pred"][:nseg]
    return pred.reshape(bs, ai, O).astype(np.float32)


_LAST_IN_MAPS = None
